# revision 17
# baseline (speedup 1.0000x reference)
"""BLiqNet (liquid-ODE MLP, single RK4 step) Trainium2 kernel — bf16 rework.

Math (reference, fp32):
    u  = x @ Wx.T + bx                  # [B, H=128]
    f(h) = -h + tanh(Wh h + Wu u + b_ode)
    RK4 dt=2 from h0=u: g_i = tanh(pre_i + b_i), with (bias-free PSUM forms)
      pre1 = (Wh+Wu) u                         b1 = (Wh+Wu)bx + b_ode
      pre2 = Wu u + Wh g1                      b2 = Wu bx + b_ode
      pre3 = pre1 + Wh (g2 - g1)               b1 again
      pre4 = (Wu-Wh) u + 2 Wh (g1 - g2 + g3)   b4 = (Wu-Wh)bx + b_ode
      h_out = (u - g1 + 2 g2 + g4)/3 (+bx/3, folded into by)
    y = h_out @ (Wout/3).T + by               by = Wout bx/3 + bout

Design (measured op costs in ns for FD=1024 on this silicon):
  - Everything bf16 on chip and on the wire (tolerance 2e-2, end-to-end
    bf16 error ~6e-3): halves HBM traffic (memory regime, per-core
    roofline ~358 GB/s) and doubles DVE throughput.
  - 24 matmuls per 1024-column tile (vs 32 for the fp32 baseline):
    pre2 computed fresh in the scratch PSUM pool while bank pair A
    carries the short accumulation pre1 -> pre3 = pre1 + Wh(g2-g1),
    which keeps the serial in-bank chain to two stages;
    bf16 and fp32r matmuls both stream 1 col/cycle (~380 ns at FD=512),
    so the win is from fewer matmuls, not the dtype.  Column halves are
    paired per weight so LDWEIGHTS amortizes.
  - ACT runs the 4 tanh (1114 each) + 1 y evacuation.
  - DVE: u/y PSUM evacuations (1212/1278) + 6 bf16 tensor_tensor
    combines (680 each, 2x mode).  scalar_tensor_tensor is avoided
    entirely (bf16 STT has no fast path: 3019); so is anything
    GPSIMD-heavy (GPSIMD's only SBUF port is DVE's second port, an
    exclusive lock, so concurrent GPS work stalls DVE tensor_tensor).
  - GPSIMD takes exactly one combine (hsum) that has a full pipeline
    stage of slack.
  - PSUM: pa pool (pre1/2/3 chain) bufs=2 + scratch pool (u, pre4, y
    rotate) bufs=2 = 8 banks exactly.  All ACT/DVE ops run at FD=1024
    to amortize the ~2.3x SBUF-access errata overhead; FD=2048 was
    tried and is a regression (DVE/GPS costs grow superlinearly there).
  - x(0) is DMA'd before the weight stack so the first matmul starts
    ~14 us in; x loads are 4 MB groups, y stores 1 MB groups, all on
    the sync HWDGE ring (issuing DMAs from the ACT ring stalls the ACT
    FIFO behind the store's semaphore wait).
"""

import sys

sys.path.insert(0, "/opt/trn_rl_repo")

import numpy as np
import ml_dtypes

from contextlib import ExitStack

import concourse.bacc as bacc
import concourse.tile as tile
from concourse import bass_utils, mybir


def _ensure_axon_hooks():
    import types
    if "antenv.axon_hooks" in sys.modules:
        return
    try:
        import antenv
        mod = types.ModuleType("antenv.axon_hooks")
        mod._hook = None
        mod.set_axon_ntff_profile_hook = lambda h: setattr(mod, "_hook", h)
        mod.get_axon_ntff_profile_hook = lambda: mod._hook
        sys.modules["antenv.axon_hooks"] = mod
        antenv.axon_hooks = mod
        try:
            if "/root/.axon_site" not in sys.path:
                sys.path.insert(0, "/root/.axon_site")
            from trn_agent_boot.trn_boot import _ntff_profile_via_ctypes
            hook = _ntff_profile_via_ctypes("/opt/axon/libaxon_pjrt.so")
            if hook is not None:
                mod.set_axon_ntff_profile_hook(hook)
        except Exception:
            pass
    except Exception:
        pass


_ensure_axon_hooks()

F32 = mybir.dt.float32
BF16 = mybir.dt.bfloat16
AF = mybir.ActivationFunctionType
ALU = mybir.AluOpType
BF_NP = ml_dtypes.bfloat16

B, D_IN, H, D_OUT = 262144, 512, 128, 256
N_CORES = 8
B_CORE = B // N_CORES
TILE = 1024

# weight-stack slots ([128, NW, 128] bf16 in DRAM)
NW = 12
WX0, WX1, WX2, WX3, SL1, SLWHN, SLWH, SL4U, SLWH2, SWO0, SWO1, SLWU = range(NW)
NB = 5  # bias columns: b1, b2, b4, by0, by1
_NC_CACHE: dict = {}


def _prep_weights(Wx, bx, Wh, Wu, b_ode, Wout, bout):
    f = np.float32
    Wx, bx, Wh, Wu = Wx.astype(f), bx.astype(f), Wh.astype(f), Wu.astype(f)
    b_ode, Wout, bout = b_ode.astype(f), Wout.astype(f), bout.astype(f)

    ws = np.zeros((128, NW, 128), dtype=f)
    WxT = Wx.T  # [512, H]
    for k in range(4):
        ws[:, WX0 + k, :] = WxT[k * 128:(k + 1) * 128, :]
    ws[:, SL1, :] = (Wh + Wu).T
    ws[:, SLWHN, :] = (-Wh).T
    ws[:, SLWH, :] = Wh.T
    ws[:, SL4U, :] = (Wu - Wh).T
    ws[:, SLWH2, :] = (2.0 * Wh).T
    ws[:, SLWU, :] = Wu.T
    WoT3 = (Wout / 3.0).T  # [128, 256]
    ws[:, SWO0, :] = WoT3[:, 0:128]
    ws[:, SWO1, :] = WoT3[:, 128:256]

    bs = np.zeros((128, NB), dtype=f)
    bs[:, 0] = (Wh + Wu) @ bx + b_ode        # b1 (pre1 and pre3)
    bs[:, 1] = Wu @ bx + b_ode               # b2
    bs[:, 2] = (Wu - Wh) @ bx + b_ode        # b4
    by = (Wout @ bx) / 3.0 + bout            # [256]
    bs[:, 3] = by[0:128]
    bs[:, 4] = by[128:256]
    return ws.astype(BF_NP), bs


# op placement per variant: engine for each of
# [ucopy, e1, ts, v, q1, gsum, q2, hsum, yev0, yev1]
# (q1 = e1 + g2, q2 = g2 + g4, hsum = q1 + q2 = u - g1 + 2 g2 + g4)
# a=ACT, d=DVE, g=GPSIMD
_PLACEMENTS = {
    "v1": "ddddd" + "gdg" + "ad",
    "v2": "ddddd" + "ddg" + "ad",   # gsum on DVE, GPS only hsum
    "v3": "adddd" + "gdg" + "dd",   # ucopy on ACT, both yev on DVE
    "v5": "ddddd" + "ddd" + "ad",   # no GPS at all
    "b1": "ddddd" + "ddg" + "an",   # yev1 on ACT on odd tiles (50%)
}


def _build(b_core: int, variant: str = "v1"):
    nc = bacc.Bacc("TRN2", target_bir_lowering=False, debug=False)

    xT_d = nc.dram_tensor("xT", [D_IN, b_core], BF16, kind="ExternalInput")
    ws_d = nc.dram_tensor("ws", [128, NW, 128], BF16, kind="ExternalInput")
    bs_d = nc.dram_tensor("bs", [128, NB], F32, kind="ExternalInput")
    yT_d = nc.dram_tensor("yT", [D_OUT, b_core], BF16, kind="ExternalOutput")

    xT_r = xT_d.rearrange("(k p) n -> p k n", p=128)  # [128, 4, b_core]
    yT_r = yT_d.rearrange("(h p) n -> p h n", p=128)  # [128, 2, b_core]

    n_tiles = b_core // TILE
    HT = TILE // 2  # 512: matmul free-dim chunk
    pl = _PLACEMENTS[variant]

    with tile.TileContext(nc) as tc, ExitStack() as ctx:
        cpool = ctx.enter_context(tc.tile_pool(name="const", bufs=1))
        xpool = ctx.enter_context(tc.tile_pool(name="x", bufs=2))
        x1pool = ctx.enter_context(tc.tile_pool(name="x1", bufs=4))
        upool = ctx.enter_context(tc.tile_pool(name="u", bufs=6))
        g1pool = ctx.enter_context(tc.tile_pool(name="g1", bufs=3))
        g2pool = ctx.enter_context(tc.tile_pool(name="g2", bufs=5))
        g3pool = ctx.enter_context(tc.tile_pool(name="g3", bufs=2))
        g4pool = ctx.enter_context(tc.tile_pool(name="g4", bufs=2))
        e1pool = ctx.enter_context(tc.tile_pool(name="e1", bufs=4))
        tspool = ctx.enter_context(tc.tile_pool(name="ts", bufs=3))
        gspool = ctx.enter_context(tc.tile_pool(name="gs", bufs=4))
        q1pool = ctx.enter_context(tc.tile_pool(name="q1", bufs=6))
        s2pool = ctx.enter_context(tc.tile_pool(name="s24", bufs=2))
        hpool = ctx.enter_context(tc.tile_pool(name="hs", bufs=4))
        ypool = ctx.enter_context(tc.tile_pool(name="y", bufs=2))

        # PSUM: pa chain pool + scratch pool, 2 bufs x 2 banks each = 8 banks
        pa_pool = ctx.enter_context(tc.tile_pool(name="pa", bufs=2, space="PSUM"))
        sc_pool = ctx.enter_context(tc.tile_pool(name="sc", bufs=2, space="PSUM"))

        # x(0) first, then weights, then x(1..2): the sync DMA queue drains
        # in order, so the first u-matmul's inputs land earliest
        _pre = []
        for i in range(4):
            xc = x1pool.tile([128, 4, TILE], BF16, tag="x1", name="x1c")
            nc.sync.dma_start(xc[:], xT_r[:, :, i * TILE:(i + 1) * TILE])
            _pre.append((xc, 0))
            if i == 0:
                ws_sb = cpool.tile([128, NW, 128], BF16)
                nc.sync.dma_start(ws_sb[:], ws_d[:])
                bs_sb = cpool.tile([128, NB], F32)
                nc.sync.dma_start(bs_sb[:], bs_d[:])

        def W(j):
            return ws_sb[:, j, :]

        def bias(j):
            return bs_sb[:, j:j + 1]

        def eng(c):
            return {"d": nc.vector, "g": nc.gpsimd}[c]

        xs, us, g1s, g2s, g3s, g4s, e1s, tss, gss, q1s, q2s, hss, ys = (
            {} for _ in range(13))
        for _i, _t in enumerate(_pre):
            xs[_i] = _t
        NPRE = 4  # tiles loaded individually up front (before ws/bs DMA)
        GRP = 4   # tiles per x-load DMA (4 MB bf16)
        YGRP = 2  # tiles per y-store DMA (1 MB bf16)

        def s0_load(i):
            if i < NPRE:
                return  # preloaded before the weight DMA below
            if (i - NPRE) % GRP:
                return
            n = min(GRP, n_tiles - i)
            xc = xpool.tile([128, 4, GRP * TILE], BF16, tag="x", name="xc")
            nc.sync.dma_start(xc[:, :, :n * TILE],
                              xT_r[:, :, i * TILE:(i + n) * TILE])
            for j in range(n):
                xs[i + j] = (xc, j * TILE)

        def s1_u(i):  # u matmuls (weight-major for LDW reuse) + evacuate
            pu = sc_pool.tile([128, TILE], F32, tag="sc", name="pu")
            xc, c0 = xs.pop(i)
            for k in range(4):
                for h in range(2):
                    nc.tensor.matmul(pu[:, h * HT:(h + 1) * HT], W(WX0 + k),
                                     xc[:, k, c0 + h * HT:c0 + (h + 1) * HT],
                                     start=(k == 0), stop=(k == 3))
            us[i] = upool.tile([128, TILE], BF16, tag="u", name="ut")
            if pl[0] == "a":
                nc.scalar.activation(us[i][:], pu[:], AF.Identity, bias=0.0)
            else:
                nc.vector.tensor_copy(us[i][:], pu[:])

        def s2_g1(i):  # pre1 = (Wh+Wu)@u in bank A ; g1 ; e1 = u - g1
            pa = pa_pool.tile([128, TILE], F32, tag="pa", name="pa")
            for h in range(2):
                nc.tensor.matmul(pa[:, h * HT:(h + 1) * HT], W(SL1),
                                 us[i][:, h * HT:(h + 1) * HT],
                                 start=True, stop=False)
            g1s[i] = g1pool.tile([128, TILE], BF16, tag="g1", name="g1t")
            nc.scalar.activation(g1s[i][:], pa[:], AF.Tanh, bias=bias(0))
            e1s[i] = e1pool.tile([128, TILE], BF16, tag="e1", name="e1t")
            eng(pl[1]).tensor_tensor(e1s[i][:], us[i][:], g1s[i][:],
                                     ALU.subtract)
            g1s[i] = (g1s[i], pa)

        def s3_g2(i):  # pre2 = Wu@u + Wh@g1 (scratch) ; g2 ; ts ; q1
            g1, pa = g1s[i]
            pb = sc_pool.tile([128, TILE], F32, tag="sc", name="pb")
            for h in range(2):
                sl = slice(h * HT, (h + 1) * HT)
                nc.tensor.matmul(pb[:, sl], W(SLWU), us[i][:, sl],
                                 start=True, stop=False)
            for h in range(2):
                sl = slice(h * HT, (h + 1) * HT)
                nc.tensor.matmul(pb[:, sl], W(SLWH), g1[:, sl],
                                 start=False, stop=True)
            g2s[i] = g2pool.tile([128, TILE], BF16, tag="g2", name="g2t")
            nc.scalar.activation(g2s[i][:], pb[:], AF.Tanh, bias=bias(1))
            tss[i] = tspool.tile([128, TILE], BF16, tag="ts", name="tst")
            eng(pl[2]).tensor_tensor(tss[i][:], g2s[i][:], g1[:], ALU.subtract)
            q1s[i] = q1pool.tile([128, TILE], BF16, tag="q1", name="q1t")
            eng(pl[4]).tensor_tensor(q1s[i][:], e1s[i][:], g2s[i][:], ALU.add)

        def s4_g3(i):  # pre3 = pre1 + Wh@ts (A bank) ; g3 ; gsum = g3 - ts
            g1, pa = g1s.pop(i)
            for h in range(2):
                sl = slice(h * HT, (h + 1) * HT)
                nc.tensor.matmul(pa[:, sl], W(SLWH), tss[i][:, sl],
                                 start=False, stop=True)
            g3s[i] = g3pool.tile([128, TILE], BF16, tag="g3", name="g3t")
            nc.scalar.activation(g3s[i][:], pa[:], AF.Tanh, bias=bias(0))
            gss[i] = gspool.tile([128, TILE], BF16, tag="gs", name="gst")
            eng(pl[5]).tensor_tensor(gss[i][:], g3s.pop(i)[:], tss.pop(i)[:],
                                     ALU.subtract)

        def s5_g4(i):  # pre4 = (Wu-Wh)@u + 2Wh@gsum ; g4 ; s24 = g2d + g4
            p4 = sc_pool.tile([128, TILE], F32, tag="sc", name="p4")
            for h in range(2):
                nc.tensor.matmul(p4[:, h * HT:(h + 1) * HT], W(SL4U),
                                 us[i][:, h * HT:(h + 1) * HT],
                                 start=True, stop=False)
            for h in range(2):
                sl = slice(h * HT, (h + 1) * HT)
                nc.tensor.matmul(p4[:, sl], W(SLWH2), gss[i][:, sl],
                                 start=False, stop=True)
            g4s[i] = g4pool.tile([128, TILE], BF16, tag="g4", name="g4t")
            nc.scalar.activation(g4s[i][:], p4[:], AF.Tanh, bias=bias(2))
            q2s[i] = s2pool.tile([128, TILE], BF16, tag="q2", name="q2t")
            eng(pl[6]).tensor_tensor(q2s[i][:], g2s.pop(i)[:],
                                     g4s.pop(i)[:], ALU.add)
            del gss[i], us[i]

        def s6_h(i):  # hsum = q1 + q2 = (u - g1 + g2) + (g2 + g4)
            hss[i] = hpool.tile([128, TILE], BF16, tag="hs", name="hst")
            eng(pl[7]).tensor_tensor(hss[i][:], q1s.pop(i)[:], q2s.pop(i)[:],
                                     ALU.add)
            del e1s[i]

        def s7_y(i):  # y halves through one scratch psum + batched store
            if i % YGRP == 0:
                ys[i // YGRP] = ypool.tile([128, 2, YGRP * TILE], BF16,
                                           tag="y", name="yt")
            y_sb = ys[i // YGRP]
            c0 = (i % YGRP) * TILE
            py = sc_pool.tile([128, TILE], F32, tag="sc", name="py")
            for fh in range(2):
                for h in range(2):
                    sl = slice(h * HT, (h + 1) * HT)
                    nc.tensor.matmul(py[:, sl], W(SWO0 + fh), hss[i][:, sl],
                                     start=True, stop=True)
                pc = pl[8 + fh]
                if pc == "m":  # mixed: ACT on ~3/8 of tiles to balance load
                    pc = "a" if (i % 8) < 3 else "d"
                elif pc == "n":  # mixed: ACT on odd tiles (50%)
                    pc = "a" if (i % 2) == 1 else "d"
                if pc == "a":
                    nc.scalar.activation(y_sb[:, fh, c0:c0 + TILE], py[:],
                                         AF.Identity, bias=bias(3 + fh))
                else:
                    nc.vector.tensor_scalar(y_sb[:, fh, c0:c0 + TILE], py[:],
                                            bias(3 + fh), None, ALU.add)
            del hss[i]
            if i % YGRP == YGRP - 1:
                g0 = (i // YGRP) * YGRP
                nc.sync.dma_start(yT_r[:, :, g0 * TILE:(g0 + YGRP) * TILE],
                                  y_sb[:])
                del ys[i // YGRP]

        # emission order within a step: s4 before s2 (pa pool rotation),
        # s7 before s5 before s1 (scratch pool rotation).  s5/s6/s7 offsets
        # leave an extra step of slack after s4/s5/s6 so the GPSIMD combines
        # (gsum, hsum) stay off the critical path.
        stages = [s0_load, s7_y, s4_g3, s5_g4, s6_h, s1_u, s2_g1, s3_g2]
        offs = [0, 10, 5, 7, 8, 2, 3, 4]
        for step in range(n_tiles + max(offs)):
            for stage, off in zip(stages, offs):
                i = step - off
                if 0 <= i < n_tiles:
                    stage(i)

    nc.compile()
    return nc


def _get_nc(b_core: int, variant: str):
    key = (b_core, variant)
    if key not in _NC_CACHE:
        _NC_CACHE[key] = _build(b_core, variant)
    return _NC_CACHE[key]


def _kernel_impl(x, Wx, bx, Wh, Wu, b_ode, Wout, bout,
                 n_cores=N_CORES, variant="v2", **run_kwargs):
    b = x.shape[0]
    b_core = b // n_cores
    ws, bs = _prep_weights(Wx, bx, Wh, Wu, b_ode, Wout, bout)

    xb = np.asarray(x, dtype=np.float32).astype(BF_NP)
    shards = xb.reshape(n_cores, b_core, D_IN).transpose(0, 2, 1)

    nc = _get_nc(b_core, variant)
    in_maps = [
        {"xT": np.ascontiguousarray(shards[c]), "ws": ws, "bs": bs}
        for c in range(n_cores)
    ]
    res = bass_utils.run_bass_kernel_spmd(
        nc, in_maps, core_ids=list(range(n_cores)), **run_kwargs
    )
    y = np.empty((b, D_OUT), dtype=np.float32)
    for c in range(n_cores):
        y[c * b_core:(c + 1) * b_core] = res.results[c]["yT"].T.astype(np.float32)
    return y, res


def kernel(x, Wx, bx, Wh, Wu, b_ode, Wout, bout):
    y, _ = _kernel_impl(x, Wx, bx, Wh, Wu, b_ode, Wout, bout)
    return y
